# revision 9
# baseline (speedup 1.0000x reference)
"""CES layer kernel for Trainium2 (8 NeuronCores, data-parallel over batch).

Computes: out[b,o] = cos(x @ angle(w).T + bias) * exp(x @ log|w|.T)
for x [262144, 128], w (real/imag) [128, 128], bias [128].

Strategy per core (B_core = 32768 rows):
  - host precomputes (tiny [128,128] work):
      L  = log|w|.T            [I, O]  (fp16)
      Th/Tl = angle(w).T/(2pi) [I, O]  (fp16 hi + fp16 lo split, "turns" units)
      b' = (bias + pi/2)/(2pi) replicated tile  (so cos(y+bias) = sin2pi(phi+b'))
  - device: SWDGE cast-DMA loads x fp32 -> fp16 SBUF (each partition reads a
    contiguous 32KB row-range; the store applies the inverse mapping); PE
    transposes x tiles (contraction dim to partitions); fp16 matmuls with xT
    stationary produce packed [mag | phi] pairs in PSUM fp32; a custom fused
    DVE op computes f = (phi + b') - round(phi + b') (exact period-1 range
    reduction); ACT evaluates exp (magnitude) and sin2pi(f) = cos(y + bias)
    from one shared activation-table set; DVE multiplies; SWDGE cast-DMA
    stores fp16 -> fp32.
"""

import os
import numpy as np

import concourse.bacc as bacc
import concourse.mybir as mybir
import concourse.hw_specs as hw_specs
import concourse.dve_ops as dve_ops
from concourse.tile import TileContext
from concourse.bass_utils import run_bass_kernel_spmd
from concourse.masks import make_identity
from concourse.dve_spec import Spec, Src0, Src1, C0

dt = mybir.dt
AF = mybir.ActivationFunctionType

B, I, O = 262144, 128, 128
N_CORES = 8
B_CORE = B // N_CORES          # 32768
CHUNK = 512                    # legacy chunk size (brep width)
BLOCK = int(os.environ.get("CES_BLOCK", "512"))  # rows per compute block
TILES_PER_BLOCK = BLOCK // 128
GROUP = int(os.environ.get("CES_GROUP", "8"))  # blocks per ACT/mul batch group
# super-chunk sizes: ramp up at the start (smaller loads land sooner, so
# compute starts earlier) and taper at the end (short serial drain).
SUPERS = [1024, 2048, 4096, 4096, 4096, 4096, 4096, 4096,
          2048, 2048, 1024]
assert sum(SUPERS) == B_CORE
TILES_PER_CHUNK = CHUNK // 128     # 4
MAGIC = float(1.5 * 2 ** 23)   # fp32 RNE round-to-integer magic constant
MUL_ON_POOL = int(os.environ.get("CES_MUL_POOL", "0"))
COPY_ACT_EVERY = int(os.environ.get("CES_COPY_ACT", "0"))

# --------------------------------------------------------------------------
# One-time in-process framework extensions (own process only; nothing on disk)
# --------------------------------------------------------------------------
_SETUP_DONE = False
ADD_FRAC_ANT = None


def _setup_framework():
    """Register the fused range-reduction DVE op and route the Sin activation
    through the hardware sin2pi table entry (shares a table set with Exp)."""
    global _SETUP_DONE, ADD_FRAC_ANT
    if _SETUP_DONE:
        return
    _y = Src0 + Src1
    ADD_FRAC_ANT = dve_ops.DveOp(
        "ADD_FRAC_ANT",
        Spec(
            body=_y - ((_y + C0) - C0),
            reference=lambda in0, in1, s0, s1, imm2: (in0 + in1)
            - ((in0 + in1 + s0) - s0),
        ),
        subdim=False,
        uops_sha={"v3": "50f66691fcf7b146", "v4": "8126f52e14432bd9"},
    )
    dve_ops.OPS.append(ADD_FRAC_ANT)
    dve_ops._SUB_OPCODE_FOR_NAME["ADD_FRAC_ANT"] = 17
    dve_ops.CUSTOM_DVE_SPECS["ADD_FRAC_ANT"] = ADD_FRAC_ANT.spec

    _real_tables = hw_specs.get_activation_tables

    def _patched_tables(arch):
        t = _real_tables(arch)
        for _name, s in t.items():
            s.discard(AF.Sin)
        t["exp_and_friends"].add(AF.Sin)
        return t

    bacc.get_activation_tables = _patched_tables
    _SETUP_DONE = True


# --------------------------------------------------------------------------
# Bass program (identical SPMD program for each core)
# --------------------------------------------------------------------------

def _patch_ldw_opt():
    import concourse.bass_utils as bu
    if getattr(bu, "_ces_ldw_patched", False):
        return
    real = bu.bir_verify_and_optimise
    def patched(*args, **kwargs):
        import unittest.mock as _mock
        real_run = bu.run_command
        def run2(argv, **kw):
            argv = ["--enable-ldw-opt=true" if a == "--enable-ldw-opt=false"
                    else a for a in argv]
            return real_run(argv, **kw)
        bu.run_command = run2
        try:
            return real(*args, **kwargs)
        finally:
            bu.run_command = real_run
    bu.bir_verify_and_optimise = patched
    bu._ces_ldw_patched = True

_NC_CACHE = None


def _build_program():
    global _NC_CACHE
    if _NC_CACHE is not None:
        return _NC_CACHE
    _setup_framework()
    if bool(int(os.environ.get('CES_LDWOPT', '0'))):
        _patch_ldw_opt()

    nc = bacc.Bacc()
    xd = nc.dram_tensor("x", [B_CORE, I], dt.float32, kind="ExternalInput")
    wcat_d = nc.dram_tensor("wcat", [I, 3 * O], dt.float16, kind="ExternalInput")
    brep_d = nc.dram_tensor("brep", [128, BLOCK], dt.float32, kind="ExternalInput")
    outd = nc.dram_tensor("out", [B_CORE, O], dt.float32, kind="ExternalOutput")

    GCH = GROUP * BLOCK  # 2048

    with TileContext(nc) as tc:
        with tc.tile_pool(name="const", bufs=1) as cpool, \
             tc.tile_pool(name="xb", bufs=int(os.environ.get("CES_XB", "4"))) as xb_pool, \
             tc.tile_pool(name="ob", bufs=3) as ob_pool, \
             tc.tile_pool(name="xt", bufs=int(os.environ.get("CES_XT", "4"))) as xt_pool, \
             tc.tile_pool(name="grp", bufs=int(os.environ.get("CES_GRP", "3"))) as gpool, \
             tc.tile_pool(name="xtps", bufs=2, space="PSUM") as xt_psum, \
             tc.tile_pool(name="mmps", bufs=3, space="PSUM") as mm_psum:

            wcat = cpool.tile([I, 3 * O], dt.float16)
            nc.sync.dma_start(out=wcat, in_=wcat_d[:, :])
            brep = cpool.tile([128, BLOCK], dt.float32)
            nc.sync.dma_start(out=brep, in_=brep_d[:, :])

            def load_super(row0, SUPER):
                xb16_full = xb_pool.tile([128, 4096], dt.float16,
                                         tag="xb16", name="xb16")
                xb16 = xb16_full[:, 0:SUPER]
                # fp32 -> fp16 cast during DMA (SWDGE); partition p reads the
                # contiguous row range [p*R, (p+1)*R) of this super.
                src = xd[row0:row0 + SUPER, :].rearrange(
                    "(p r) i -> p r i", p=128)
                nc.gpsimd.dma_start(
                    out=xb16.rearrange("p (r i) -> p r i", i=128), in_=src)
                return xb16

            # issue the first load before identity setup so the first x
            # transfer hits the DMA engines as early as possible
            xb_first = load_super(0, SUPERS[0])
            ident = cpool.tile([128, 128], dt.float16)
            make_identity(nc, ident)

            # sin/mul (and the store that consumes them) are emitted one
            # group BEHIND the matmul/frac/exp stream: the scalar engine then
            # fills the "waiting for the group's last frac" window with the
            # next group's exps instead of idling.
            pending = []   # list of closures to emit, one group behind

            def flush_pending():
                for fn in pending:
                    fn()
                pending.clear()

            row0 = 0
            for s, SUPER in enumerate(SUPERS):
                xb16 = xb_first if s == 0 else load_super(row0, SUPER)

                out16_full = ob_pool.tile([128, 4096], dt.float16,
                                          tag="out16", name="out16")
                out16 = out16_full[:, 0:SUPER]

                BLOCKS_PER_SUPER = SUPER // BLOCK
                groups = []
                b0 = 0
                while b0 < BLOCKS_PER_SUPER:
                    gsz = min(GROUP, BLOCKS_PER_SUPER - b0)
                    groups.append((b0, gsz))
                    b0 += gsz
                for gi, (gstart, GSZ) in enumerate(groups):
                    f4_full = gpool.tile([128, GCH], dt.float32, tag="f4",
                                         name="f4")
                    f4 = f4_full[:, 0:GSZ * BLOCK]
                    mag4_full = gpool.tile([128, GCH], dt.float16, tag="mag4",
                                           name="mag4")
                    mag4 = mag4_full[:, 0:GSZ * BLOCK]
                    cosy4_full = gpool.tile([128, GCH], dt.float16,
                                            tag="cosy4", name="cosy4")
                    cosy4 = cosy4_full[:, 0:GSZ * BLOCK]

                    for cg in range(GSZ):
                        blk = gstart + cg
                        col0 = blk * BLOCK
                        gcol0 = cg * BLOCK
                        # --- transpose tiles: [b,i] -> [i,b] fp16 PSUM ---
                        xt_ps = xt_psum.tile([128, BLOCK], dt.float16, tag="xt")
                        for t in range(TILES_PER_BLOCK):
                            nc.tensor.transpose(
                                xt_ps[:, t * 128:(t + 1) * 128],
                                xb16[:, col0 + t * 128: col0 + (t + 1) * 128],
                                ident,
                            )
                        xt_sb = xt_pool.tile([128, BLOCK], dt.float16,
                                             tag="xt_sb")
                        if COPY_ACT_EVERY and blk % COPY_ACT_EVERY == 0:
                            nc.scalar.copy(xt_sb, xt_ps)
                        else:
                            nc.vector.tensor_copy(xt_sb, xt_ps)

                        # --- matmuls: [mag | phi_h] then phi_l accumulate ---
                        mm_ps = mm_psum.tile([128, TILES_PER_BLOCK * 256],
                                             dt.float32, tag="mm")
                        for t in range(TILES_PER_BLOCK):
                            lhsT = xt_sb[:, t * 128:(t + 1) * 128]
                            base = t * 256
                            nc.tensor.matmul(
                                mm_ps[:, base:base + 256], lhsT,
                                wcat[:, 0:256], start=True, stop=False)
                            nc.tensor.matmul(
                                mm_ps[:, base + 128:base + 256], lhsT,
                                wcat[:, 256:384], start=False, stop=True)

                        mm3 = mm_ps.rearrange("p (t n) -> p t n", n=256)
                        # --- fused bias + period-1 range reduction (DVE) ---
                        nc.vector._custom_dve(
                            ADD_FRAC_ANT,
                            out=f4[:, gcol0:gcol0 + BLOCK].rearrange(
                                "p (t n) -> p t n", n=128),
                            in0=mm3[:, :, 128:256],
                            in1=brep.rearrange("p (t n) -> p t n", n=128),
                            s0=MAGIC,
                        )
                        # --- magnitude: exp (ACT, psum src) ---
                        nc.scalar.activation(
                            mag4[:, gcol0:gcol0 + BLOCK].rearrange(
                                "p (t n) -> p t n", n=128),
                            mm3[:, :, 0:128], AF.Exp, bias=0.0, scale=1.0)

                    # --- batched sin2pi + multiply, pipelined one group back
                    prev = pending[:]
                    pending.clear()

                    def tail(cosy4=cosy4, f4=f4, mag4=mag4, out16=out16,
                             gstart=gstart, GSZ=GSZ):
                        nc.scalar.activation(cosy4, f4, AF.Sin,
                                             bias=0.0, scale=1.0)  # -> sin2pi
                        nc.vector.tensor_mul(
                            out16[:, gstart * BLOCK:(gstart + GSZ) * BLOCK],
                            mag4, cosy4)
                    pending.append(tail)
                    if gi == len(groups) - 1:
                        def store(out16=out16, row0=row0, SUPER=SUPER):
                            dst = outd[row0:row0 + SUPER, :].rearrange(
                                "(p r) i -> p r i", p=128)
                            nc.gpsimd.dma_start(
                                out=dst,
                                in_=out16.rearrange("p (r i) -> p r i", i=128))
                        pending.append(store)
                    for fn in prev:
                        fn()

                row0 += SUPER
            flush_pending()

    nc.compile()

    _real_tjb = nc.to_json_bytes
    nc.to_json_bytes = lambda: _real_tjb().replace(b'"func":"Sin"',
                                                   b'"func":"Sin2pi"')
    _NC_CACHE = nc
    return nc


# --------------------------------------------------------------------------
# Host-side entry point
# --------------------------------------------------------------------------
LAST_RESULT = None


def kernel(x, w_real, w_imag, bias):
    global LAST_RESULT
    x = np.ascontiguousarray(np.asarray(x, dtype=np.float32))
    w_real = np.asarray(w_real, dtype=np.float32)
    w_imag = np.asarray(w_imag, dtype=np.float32)
    bias = np.asarray(bias, dtype=np.float32)

    wr = w_real.astype(np.float64)
    wi = w_imag.astype(np.float64)
    L = 0.5 * np.log(wr * wr + wi * wi)            # [O, I] log|w|
    T = np.arctan2(wi, wr) / (2 * np.pi)           # [O, I] angle in turns
    LT = np.ascontiguousarray(L.T)                 # [I, O]
    TT = np.ascontiguousarray(T.T)                 # [I, O]
    Lh = LT.astype(np.float16)
    Th = TT.astype(np.float16)
    Tl = (TT - Th.astype(np.float64)).astype(np.float16)
    wcat = np.concatenate([Lh, Th, Tl], axis=1)    # [I, 3*O] fp16
    bp = ((bias.astype(np.float64) + np.pi / 2) / (2 * np.pi)).astype(np.float32)
    brep = np.broadcast_to(
        np.tile(bp, BLOCK // O)[None, :], (128, BLOCK)).copy()

    nc = _build_program()

    in_maps = []
    for c in range(N_CORES):
        in_maps.append({
            "x": x[c * B_CORE:(c + 1) * B_CORE, :],
            "wcat": wcat,
            "brep": brep,
        })

    trace = bool(int(os.environ.get("CES_TRACE", "0")))
    if trace:
        _install_trace_shim()
    # Retry once on transient device faults (a wedged NeuronCore usually
    # recovers on the next execute).
    try:
        res = run_bass_kernel_spmd(nc, in_maps, core_ids=list(range(N_CORES)),
                                   trace=trace)
    except Exception:
        import time
        time.sleep(2.0)
        res = run_bass_kernel_spmd(nc, in_maps, core_ids=list(range(N_CORES)),
                                   trace=False)
    LAST_RESULT = res
    if trace and res.exec_time_ns is not None:
        print(f"HW exec time: {res.exec_time_ns} ns", flush=True)

    out = np.empty((B, O), dtype=np.float32)
    for c in range(N_CORES):
        out[c * B_CORE:(c + 1) * B_CORE, :] = res.results[c]["out"]
    return out


def _install_trace_shim():
    """NTFF profiling hook shim (this image's antenv lacks axon_hooks)."""
    import sys
    import types
    import importlib.util as ilu
    if ilu.find_spec("antenv.axon_hooks") is None and \
            "antenv.axon_hooks" not in sys.modules:
        m = types.ModuleType("antenv.axon_hooks")
        h = [None]
        m.set_axon_ntff_profile_hook = lambda v: h.__setitem__(0, v)
        m.get_axon_ntff_profile_hook = lambda: h[0]
        sys.modules["antenv.axon_hooks"] = m
        sys.path.insert(0, "/root/.axon_site")
        try:
            from trn_agent_boot.trn_boot import _ntff_profile_via_ctypes
            m.set_axon_ntff_profile_hook(
                _ntff_profile_via_ctypes("/opt/axon/libaxon_pjrt.so"))
        except Exception:
            pass
    import concourse.bass_utils as bu
    bu.upload_artifacts = lambda d: "local://skipped"


if __name__ == "__main__":
    rng = np.random.default_rng(0)
    x = rng.uniform(-1, 1, (B, I)).astype(np.float32)
    th = rng.uniform(-np.pi, np.pi, (O, I)).astype(np.float32)
    sc = np.exp(0.1 * rng.standard_normal((O, I))).astype(np.float32)
    wr = sc * np.cos(th)
    wi = sc * np.sin(th)
    bs = rng.uniform(-np.pi, np.pi, (O,)).astype(np.float32)
    got = kernel(x, wr, wi, bs)
    absw = np.sqrt(wr.astype(np.float64)**2 + wi.astype(np.float64)**2)
    angw = np.arctan2(wi.astype(np.float64), wr.astype(np.float64))
    mag = np.exp(x.astype(np.float64) @ np.log(absw).T)
    y = x.astype(np.float64) @ angw.T + bs
    ref = np.cos(y) * mag
    err = np.abs(got - ref)
    print(f"absmax={err.max():.3e} scale={np.abs(ref).max():.2f} "
          f"absmax/scale={err.max()/np.abs(ref).max():.3e} "
          f"relL2={np.linalg.norm(got-ref)/np.linalg.norm(ref):.3e}")



# revision 13
# speedup vs baseline: 1.0642x; 1.0642x over previous
"""CES layer kernel for Trainium2 (8 NeuronCores, data-parallel over batch).

Computes: out[b,o] = cos(x @ angle(w).T + bias) * exp(x @ log|w|.T)
for x [262144, 128], w (real/imag) [128, 128], bias [128].

Strategy per core (B_core = 32768 rows):
  - host precomputes (tiny [128,128] work):
      L  = log|w|.T            [I, O]  (fp16)
      Th/Tl = angle(w).T/(2pi) [I, O]  (fp16 hi + fp16 lo split, "turns" units)
      b' = (bias + pi/2)/(2pi) replicated tile  (so cos(y+bias) = sin2pi(phi+b'))
  - device: SWDGE cast-DMA loads x fp32 -> fp16 SBUF (each partition reads a
    contiguous 32KB row-range; the store applies the inverse mapping); PE
    transposes x tiles (contraction dim to partitions); fp16 matmuls with xT
    stationary produce packed [mag | phi] pairs in PSUM fp32; a custom fused
    DVE op computes f = (phi + b') - round(phi + b') (exact period-1 range
    reduction); ACT evaluates exp (magnitude) and sin2pi(f) = cos(y + bias)
    from one shared activation-table set; DVE multiplies; SWDGE cast-DMA
    stores fp16 -> fp32.
"""

import os
import numpy as np

import concourse.bacc as bacc
import concourse.mybir as mybir
import concourse.hw_specs as hw_specs
import concourse.dve_ops as dve_ops
from concourse.tile import TileContext
from concourse.bass_utils import run_bass_kernel_spmd
from concourse.masks import make_identity
from concourse.dve_spec import Spec, Src0, Src1, C0

dt = mybir.dt
AF = mybir.ActivationFunctionType

B, I, O = 262144, 128, 128
N_CORES = 8
B_CORE = B // N_CORES          # 32768
CHUNK = 512                    # legacy chunk size (brep width)
BLOCK = int(os.environ.get("CES_BLOCK", "512"))  # rows per compute block
TILES_PER_BLOCK = BLOCK // 128
GROUP = int(os.environ.get("CES_GROUP", "8"))  # blocks per ACT/mul batch group
# super-chunk sizes: small at the ends to shorten pipeline fill/drain.
# Same transfer-size multiset as the original schedule, but the smallest
# super goes FIRST so the first load lands (and compute starts) earlier.
SUPERS = [1024, 2048] + [4096] * 6 + [2048, 2048, 1024]
assert sum(SUPERS) == B_CORE
TILES_PER_CHUNK = CHUNK // 128     # 4
MAGIC = float(1.5 * 2 ** 23)   # fp32 RNE round-to-integer magic constant
MUL_ON_POOL = int(os.environ.get("CES_MUL_POOL", "0"))
COPY_ACT_EVERY = int(os.environ.get("CES_COPY_ACT", "0"))

# --------------------------------------------------------------------------
# One-time in-process framework extensions (own process only; nothing on disk)
# --------------------------------------------------------------------------
_SETUP_DONE = False
ADD_FRAC_ANT = None


def _setup_framework():
    """Register the fused range-reduction DVE op and route the Sin activation
    through the hardware sin2pi table entry (shares a table set with Exp)."""
    global _SETUP_DONE, ADD_FRAC_ANT
    if _SETUP_DONE:
        return
    _y = Src0 + Src1
    ADD_FRAC_ANT = dve_ops.DveOp(
        "ADD_FRAC_ANT",
        Spec(
            body=_y - ((_y + C0) - C0),
            reference=lambda in0, in1, s0, s1, imm2: (in0 + in1)
            - ((in0 + in1 + s0) - s0),
        ),
        subdim=False,
        uops_sha={"v3": "50f66691fcf7b146", "v4": "8126f52e14432bd9"},
    )
    dve_ops.OPS.append(ADD_FRAC_ANT)
    dve_ops._SUB_OPCODE_FOR_NAME["ADD_FRAC_ANT"] = 17
    dve_ops.CUSTOM_DVE_SPECS["ADD_FRAC_ANT"] = ADD_FRAC_ANT.spec

    _real_tables = hw_specs.get_activation_tables

    def _patched_tables(arch):
        t = _real_tables(arch)
        for _name, s in t.items():
            s.discard(AF.Sin)
        t["exp_and_friends"].add(AF.Sin)
        return t

    bacc.get_activation_tables = _patched_tables
    _SETUP_DONE = True


# --------------------------------------------------------------------------
# Bass program (identical SPMD program for each core)
# --------------------------------------------------------------------------

def _patch_ldw_opt():
    import concourse.bass_utils as bu
    if getattr(bu, "_ces_ldw_patched", False):
        return
    real = bu.bir_verify_and_optimise
    def patched(*args, **kwargs):
        import unittest.mock as _mock
        real_run = bu.run_command
        def run2(argv, **kw):
            argv = ["--enable-ldw-opt=true" if a == "--enable-ldw-opt=false"
                    else a for a in argv]
            return real_run(argv, **kw)
        bu.run_command = run2
        try:
            return real(*args, **kwargs)
        finally:
            bu.run_command = real_run
    bu.bir_verify_and_optimise = patched
    bu._ces_ldw_patched = True

_NC_CACHE = None


def _build_program():
    global _NC_CACHE
    if _NC_CACHE is not None:
        return _NC_CACHE
    _setup_framework()
    if bool(int(os.environ.get('CES_LDWOPT', '0'))):
        _patch_ldw_opt()

    nc = bacc.Bacc()
    xd = nc.dram_tensor("x", [B_CORE, I], dt.float32, kind="ExternalInput")
    wcat_d = nc.dram_tensor("wcat", [I, 3 * O], dt.float16, kind="ExternalInput")
    brep_d = nc.dram_tensor("brep", [128, BLOCK], dt.float32, kind="ExternalInput")
    outd = nc.dram_tensor("out", [B_CORE, O], dt.float32, kind="ExternalOutput")

    GCH = GROUP * BLOCK  # 2048

    with TileContext(nc) as tc:
        with tc.tile_pool(name="const", bufs=1) as cpool, \
             tc.tile_pool(name="xb", bufs=int(os.environ.get("CES_XB", "4"))) as xb_pool, \
             tc.tile_pool(name="ob", bufs=3) as ob_pool, \
             tc.tile_pool(name="xt", bufs=int(os.environ.get("CES_XT", "4"))) as xt_pool, \
             tc.tile_pool(name="grp", bufs=int(os.environ.get("CES_GRP", "3"))) as gpool, \
             tc.tile_pool(name="xtps", bufs=2, space="PSUM") as xt_psum, \
             tc.tile_pool(name="mmps", bufs=3, space="PSUM") as mm_psum:

            wcat = cpool.tile([I, 3 * O], dt.float16)
            nc.sync.dma_start(out=wcat, in_=wcat_d[:, :])
            brep = cpool.tile([128, BLOCK], dt.float32)
            nc.sync.dma_start(out=brep, in_=brep_d[:, :])
            ident = cpool.tile([128, 128], dt.float16)
            make_identity(nc, ident)

            row0 = 0
            for s, SUPER in enumerate(SUPERS):
                CHUNKS_PER_SUPER = SUPER // CHUNK
                xb16_full = xb_pool.tile([128, 4096], dt.float16,
                                         tag="xb16", name="xb16")
                xb16 = xb16_full[:, 0:SUPER]
                # fp32 -> fp16 cast during DMA (SWDGE); partition p reads the
                # contiguous row range [p*R, (p+1)*R) of this super.
                src = xd[row0:row0 + SUPER, :].rearrange(
                    "(p r) i -> p r i", p=128)
                nc.gpsimd.dma_start(
                    out=xb16.rearrange("p (r i) -> p r i", i=128), in_=src)

                out16_full = ob_pool.tile([128, 4096], dt.float16,
                                          tag="out16", name="out16")
                out16 = out16_full[:, 0:SUPER]

                BLOCKS_PER_SUPER = SUPER // BLOCK
                GSZ = min(GROUP, BLOCKS_PER_SUPER)   # blocks per sin/mul batch
                for g in range(BLOCKS_PER_SUPER // GSZ):
                    f4_full = gpool.tile([128, GCH], dt.float32, tag="f4",
                                         name="f4")
                    f4 = f4_full[:, 0:GSZ * BLOCK]
                    mag4_full = gpool.tile([128, GCH], dt.float16, tag="mag4",
                                           name="mag4")
                    mag4 = mag4_full[:, 0:GSZ * BLOCK]
                    cosy4_full = gpool.tile([128, GCH], dt.float16,
                                            tag="cosy4", name="cosy4")
                    cosy4 = cosy4_full[:, 0:GSZ * BLOCK]

                    for cg in range(GSZ):
                        blk = g * GSZ + cg
                        col0 = blk * BLOCK
                        gcol0 = cg * BLOCK
                        # --- transpose tiles: [b,i] -> [i,b] fp16 PSUM ---
                        xt_ps = xt_psum.tile([128, BLOCK], dt.float16, tag="xt")
                        for t in range(TILES_PER_BLOCK):
                            nc.tensor.transpose(
                                xt_ps[:, t * 128:(t + 1) * 128],
                                xb16[:, col0 + t * 128: col0 + (t + 1) * 128],
                                ident,
                            )
                        xt_sb = xt_pool.tile([128, BLOCK], dt.float16,
                                             tag="xt_sb")
                        if COPY_ACT_EVERY and blk % COPY_ACT_EVERY == 0:
                            nc.scalar.copy(xt_sb, xt_ps)
                        else:
                            nc.vector.tensor_copy(xt_sb, xt_ps)

                        # --- matmuls: [mag | phi_h] then phi_l accumulate ---
                        mm_ps = mm_psum.tile([128, TILES_PER_BLOCK * 256],
                                             dt.float32, tag="mm")
                        for t in range(TILES_PER_BLOCK):
                            lhsT = xt_sb[:, t * 128:(t + 1) * 128]
                            base = t * 256
                            nc.tensor.matmul(
                                mm_ps[:, base:base + 256], lhsT,
                                wcat[:, 0:256], start=True, stop=False)
                            nc.tensor.matmul(
                                mm_ps[:, base + 128:base + 256], lhsT,
                                wcat[:, 256:384], start=False, stop=True)

                        mm3 = mm_ps.rearrange("p (t n) -> p t n", n=256)
                        # --- fused bias + period-1 range reduction (DVE) ---
                        nc.vector._custom_dve(
                            ADD_FRAC_ANT,
                            out=f4[:, gcol0:gcol0 + BLOCK].rearrange(
                                "p (t n) -> p t n", n=128),
                            in0=mm3[:, :, 128:256],
                            in1=brep.rearrange("p (t n) -> p t n", n=128),
                            s0=MAGIC,
                        )
                        # --- magnitude: exp (ACT, psum src) ---
                        nc.scalar.activation(
                            mag4[:, gcol0:gcol0 + BLOCK].rearrange(
                                "p (t n) -> p t n", n=128),
                            mm3[:, :, 0:128], AF.Exp, bias=0.0, scale=1.0)

                    # --- batched sin2pi + multiply over the group ---
                    # sin is split in two halves: the first half's fracs are
                    # long done when ACT reaches it, and evaluating it covers
                    # the window in which DVE finishes the second half's
                    # fracs — removing a ~1.1us/group ACT stall.
                    gw = GSZ * BLOCK
                    if gw >= 2048:
                        nc.scalar.activation(cosy4[:, 0:gw // 2],
                                             f4[:, 0:gw // 2], AF.Sin,
                                             bias=0.0, scale=1.0)
                        nc.scalar.activation(cosy4[:, gw // 2:gw],
                                             f4[:, gw // 2:gw], AF.Sin,
                                             bias=0.0, scale=1.0)
                    else:
                        nc.scalar.activation(cosy4, f4, AF.Sin,
                                             bias=0.0, scale=1.0)  # -> sin2pi
                    if MUL_ON_POOL == 2:
                        eng = nc.gpsimd if (g % 2 == 0) else nc.vector
                    elif MUL_ON_POOL == 1:
                        eng = nc.gpsimd
                    else:
                        eng = nc.vector
                    eng.tensor_mul(
                        out16[:, g * GSZ * BLOCK:(g + 1) * GSZ * BLOCK],
                        mag4, cosy4)

                dst = outd[row0:row0 + SUPER, :].rearrange(
                    "(p r) i -> p r i", p=128)
                nc.gpsimd.dma_start(
                    out=dst, in_=out16.rearrange("p (r i) -> p r i", i=128))
                row0 += SUPER

    nc.compile()

    _real_tjb = nc.to_json_bytes
    nc.to_json_bytes = lambda: _real_tjb().replace(b'"func":"Sin"',
                                                   b'"func":"Sin2pi"')
    _NC_CACHE = nc
    return nc


# --------------------------------------------------------------------------
# Host-side entry point
# --------------------------------------------------------------------------
LAST_RESULT = None


def kernel(x, w_real, w_imag, bias):
    global LAST_RESULT
    x = np.ascontiguousarray(np.asarray(x, dtype=np.float32))
    w_real = np.asarray(w_real, dtype=np.float32)
    w_imag = np.asarray(w_imag, dtype=np.float32)
    bias = np.asarray(bias, dtype=np.float32)

    wr = w_real.astype(np.float64)
    wi = w_imag.astype(np.float64)
    L = 0.5 * np.log(wr * wr + wi * wi)            # [O, I] log|w|
    T = np.arctan2(wi, wr) / (2 * np.pi)           # [O, I] angle in turns
    LT = np.ascontiguousarray(L.T)                 # [I, O]
    TT = np.ascontiguousarray(T.T)                 # [I, O]
    Lh = LT.astype(np.float16)
    Th = TT.astype(np.float16)
    Tl = (TT - Th.astype(np.float64)).astype(np.float16)
    wcat = np.concatenate([Lh, Th, Tl], axis=1)    # [I, 3*O] fp16
    bp = ((bias.astype(np.float64) + np.pi / 2) / (2 * np.pi)).astype(np.float32)
    brep = np.broadcast_to(
        np.tile(bp, BLOCK // O)[None, :], (128, BLOCK)).copy()

    nc = _build_program()

    in_maps = []
    for c in range(N_CORES):
        in_maps.append({
            "x": x[c * B_CORE:(c + 1) * B_CORE, :],
            "wcat": wcat,
            "brep": brep,
        })

    trace = bool(int(os.environ.get("CES_TRACE", "0")))
    if trace:
        _install_trace_shim()
    # Retry once on transient device faults (a wedged NeuronCore usually
    # recovers on the next execute).
    try:
        res = run_bass_kernel_spmd(nc, in_maps, core_ids=list(range(N_CORES)),
                                   trace=trace)
    except Exception:
        import time
        time.sleep(2.0)
        res = run_bass_kernel_spmd(nc, in_maps, core_ids=list(range(N_CORES)),
                                   trace=False)
    LAST_RESULT = res
    if trace and res.exec_time_ns is not None:
        print(f"HW exec time: {res.exec_time_ns} ns", flush=True)

    out = np.empty((B, O), dtype=np.float32)
    for c in range(N_CORES):
        out[c * B_CORE:(c + 1) * B_CORE, :] = res.results[c]["out"]
    return out


def _install_trace_shim():
    """NTFF profiling hook shim (this image's antenv lacks axon_hooks)."""
    import sys
    import types
    import importlib.util as ilu
    if ilu.find_spec("antenv.axon_hooks") is None and \
            "antenv.axon_hooks" not in sys.modules:
        m = types.ModuleType("antenv.axon_hooks")
        h = [None]
        m.set_axon_ntff_profile_hook = lambda v: h.__setitem__(0, v)
        m.get_axon_ntff_profile_hook = lambda: h[0]
        sys.modules["antenv.axon_hooks"] = m
        sys.path.insert(0, "/root/.axon_site")
        try:
            from trn_agent_boot.trn_boot import _ntff_profile_via_ctypes
            m.set_axon_ntff_profile_hook(
                _ntff_profile_via_ctypes("/opt/axon/libaxon_pjrt.so"))
        except Exception:
            pass
    import concourse.bass_utils as bu
    bu.upload_artifacts = lambda d: "local://skipped"


if __name__ == "__main__":
    rng = np.random.default_rng(0)
    x = rng.uniform(-1, 1, (B, I)).astype(np.float32)
    th = rng.uniform(-np.pi, np.pi, (O, I)).astype(np.float32)
    sc = np.exp(0.1 * rng.standard_normal((O, I))).astype(np.float32)
    wr = sc * np.cos(th)
    wi = sc * np.sin(th)
    bs = rng.uniform(-np.pi, np.pi, (O,)).astype(np.float32)
    got = kernel(x, wr, wi, bs)
    absw = np.sqrt(wr.astype(np.float64)**2 + wi.astype(np.float64)**2)
    angw = np.arctan2(wi.astype(np.float64), wr.astype(np.float64))
    mag = np.exp(x.astype(np.float64) @ np.log(absw).T)
    y = x.astype(np.float64) @ angw.T + bs
    ref = np.cos(y) * mag
    err = np.abs(got - ref)
    print(f"absmax={err.max():.3e} scale={np.abs(ref).max():.2f} "
          f"absmax/scale={err.max()/np.abs(ref).max():.3e} "
          f"relL2={np.linalg.norm(got-ref)/np.linalg.norm(ref):.3e}")



# revision 14
# speedup vs baseline: 1.1608x; 1.0908x over previous
"""CES layer kernel for Trainium2 (8 NeuronCores, data-parallel over batch).

Computes: out[b,o] = cos(x @ angle(w).T + bias) * exp(x @ log|w|.T)
for x [262144, 128], w (real/imag) [128, 128], bias [128].

Strategy per core (B_core = 32768 rows):
  - host precomputes (tiny [128,128] work):
      L  = log|w|.T            [I, O]  (fp16)
      Th/Tl = angle(w).T/(2pi) [I, O]  (fp16 hi + fp16 lo split, "turns" units)
      b' = (bias + pi/2)/(2pi) replicated tile  (so cos(y+bias) = sin2pi(phi+b'))
  - device: SWDGE cast-DMA loads x fp32 -> fp16 SBUF (each partition reads a
    contiguous 32KB row-range; the store applies the inverse mapping); PE
    transposes x tiles (contraction dim to partitions); fp16 matmuls with xT
    stationary produce packed [mag | phi] pairs in PSUM fp32; a custom fused
    DVE op computes f = (phi + b') - round(phi + b') (exact period-1 range
    reduction); ACT evaluates exp (magnitude) and sin2pi(f) = cos(y + bias)
    from one shared activation-table set; DVE multiplies; SWDGE cast-DMA
    stores fp16 -> fp32.
"""

import os
import numpy as np

import concourse.bacc as bacc
import concourse.mybir as mybir
import concourse.hw_specs as hw_specs
import concourse.dve_ops as dve_ops
from concourse.tile import TileContext
from concourse.bass_utils import run_bass_kernel_spmd
from concourse.masks import make_identity
from concourse.dve_spec import Spec, Src0, Src1, C0

dt = mybir.dt
AF = mybir.ActivationFunctionType

B, I, O = 262144, 128, 128
N_CORES = 8
B_CORE = B // N_CORES          # 32768
CHUNK = 512                    # legacy chunk size (brep width)
BLOCK = int(os.environ.get("CES_BLOCK", "512"))  # rows per compute block
TILES_PER_BLOCK = BLOCK // 128
GROUP = int(os.environ.get("CES_GROUP", "8"))  # blocks per ACT/mul batch group
# super-chunk sizes: small at the ends to shorten pipeline fill/drain
SUPERS = [2048, 2048] + [4096] * 6 + [2048, 1024, 1024]
assert sum(SUPERS) == B_CORE
TILES_PER_CHUNK = CHUNK // 128     # 4
MAGIC = float(1.5 * 2 ** 23)   # fp32 RNE round-to-integer magic constant
MUL_ON_POOL = int(os.environ.get("CES_MUL_POOL", "0"))
COPY_ACT_EVERY = int(os.environ.get("CES_COPY_ACT", "0"))

# --------------------------------------------------------------------------
# One-time in-process framework extensions (own process only; nothing on disk)
# --------------------------------------------------------------------------
_SETUP_DONE = False
ADD_FRAC_ANT = None


def _setup_framework():
    """Register the fused range-reduction DVE op and route the Sin activation
    through the hardware sin2pi table entry (shares a table set with Exp)."""
    global _SETUP_DONE, ADD_FRAC_ANT
    if _SETUP_DONE:
        return
    _y = Src0 + Src1
    ADD_FRAC_ANT = dve_ops.DveOp(
        "ADD_FRAC_ANT",
        Spec(
            body=_y - ((_y + C0) - C0),
            reference=lambda in0, in1, s0, s1, imm2: (in0 + in1)
            - ((in0 + in1 + s0) - s0),
        ),
        subdim=False,
        uops_sha={"v3": "50f66691fcf7b146", "v4": "8126f52e14432bd9"},
    )
    dve_ops.OPS.append(ADD_FRAC_ANT)
    dve_ops._SUB_OPCODE_FOR_NAME["ADD_FRAC_ANT"] = 17
    dve_ops.CUSTOM_DVE_SPECS["ADD_FRAC_ANT"] = ADD_FRAC_ANT.spec

    _real_tables = hw_specs.get_activation_tables

    def _patched_tables(arch):
        t = _real_tables(arch)
        for _name, s in t.items():
            s.discard(AF.Sin)
        t["exp_and_friends"].add(AF.Sin)
        return t

    bacc.get_activation_tables = _patched_tables
    _SETUP_DONE = True


# --------------------------------------------------------------------------
# Bass program (identical SPMD program for each core)
# --------------------------------------------------------------------------

def _patch_ldw_opt():
    import concourse.bass_utils as bu
    if getattr(bu, "_ces_ldw_patched", False):
        return
    real = bu.bir_verify_and_optimise
    def patched(*args, **kwargs):
        import unittest.mock as _mock
        real_run = bu.run_command
        def run2(argv, **kw):
            argv = ["--enable-ldw-opt=true" if a == "--enable-ldw-opt=false"
                    else a for a in argv]
            return real_run(argv, **kw)
        bu.run_command = run2
        try:
            return real(*args, **kwargs)
        finally:
            bu.run_command = real_run
    bu.bir_verify_and_optimise = patched
    bu._ces_ldw_patched = True

_NC_CACHE = None


def _build_program():
    global _NC_CACHE
    if _NC_CACHE is not None:
        return _NC_CACHE
    _setup_framework()
    if bool(int(os.environ.get('CES_LDWOPT', '0'))):
        _patch_ldw_opt()

    nc = bacc.Bacc()
    xd = nc.dram_tensor("x", [B_CORE, I], dt.float32, kind="ExternalInput")
    wcat_d = nc.dram_tensor("wcat", [I, 3 * O], dt.float16, kind="ExternalInput")
    brep_d = nc.dram_tensor("brep", [128, BLOCK], dt.float32, kind="ExternalInput")
    outd = nc.dram_tensor("out", [B_CORE, O], dt.float32, kind="ExternalOutput")

    GCH = GROUP * BLOCK  # 2048

    with TileContext(nc) as tc:
        with tc.tile_pool(name="const", bufs=1) as cpool, \
             tc.tile_pool(name="xb", bufs=int(os.environ.get("CES_XB", "3"))) as xb_pool, \
             tc.tile_pool(name="ob", bufs=3) as ob_pool, \
             tc.tile_pool(name="xt", bufs=int(os.environ.get("CES_XT", "4"))) as xt_pool, \
             tc.tile_pool(name="grp", bufs=int(os.environ.get("CES_GRP", "3"))) as gpool, \
             tc.tile_pool(name="xtps", bufs=2, space="PSUM") as xt_psum, \
             tc.tile_pool(name="mmps", bufs=3, space="PSUM") as mm_psum:

            wcat = cpool.tile([I, 3 * O], dt.float16)
            nc.sync.dma_start(out=wcat, in_=wcat_d[:, :])
            brep = cpool.tile([128, BLOCK], dt.float32)
            nc.sync.dma_start(out=brep, in_=brep_d[:, :])
            ident = cpool.tile([128, 128], dt.float16)
            make_identity(nc, ident)

            row0 = 0
            for s, SUPER in enumerate(SUPERS):
                CHUNKS_PER_SUPER = SUPER // CHUNK
                xb16_full = xb_pool.tile([128, 4096], dt.float16,
                                         tag="xb16", name="xb16")
                xb16 = xb16_full[:, 0:SUPER]
                # fp32 -> fp16 cast during DMA (SWDGE); partition p reads the
                # contiguous row range [p*R, (p+1)*R) of this super.
                src = xd[row0:row0 + SUPER, :].rearrange(
                    "(p r) i -> p r i", p=128)
                nc.gpsimd.dma_start(
                    out=xb16.rearrange("p (r i) -> p r i", i=128), in_=src)

                out16_full = ob_pool.tile([128, 4096], dt.float16,
                                          tag="out16", name="out16")
                out16 = out16_full[:, 0:SUPER]

                BLOCKS_PER_SUPER = SUPER // BLOCK
                GSZ = min(GROUP, BLOCKS_PER_SUPER)   # blocks per sin/mul batch
                for g in range(BLOCKS_PER_SUPER // GSZ):
                    f4_full = gpool.tile([128, GCH], dt.float32, tag="f4",
                                         name="f4")
                    f4 = f4_full[:, 0:GSZ * BLOCK]
                    mag4_full = gpool.tile([128, GCH], dt.float16, tag="mag4",
                                           name="mag4")
                    mag4 = mag4_full[:, 0:GSZ * BLOCK]
                    cosy4_full = gpool.tile([128, GCH], dt.float16,
                                            tag="cosy4", name="cosy4")
                    cosy4 = cosy4_full[:, 0:GSZ * BLOCK]

                    for cg in range(GSZ):
                        blk = g * GSZ + cg
                        col0 = blk * BLOCK
                        gcol0 = cg * BLOCK
                        # --- transpose tiles: [b,i] -> [i,b] fp16 PSUM ---
                        xt_ps = xt_psum.tile([128, BLOCK], dt.float16, tag="xt")
                        for t in range(TILES_PER_BLOCK):
                            nc.tensor.transpose(
                                xt_ps[:, t * 128:(t + 1) * 128],
                                xb16[:, col0 + t * 128: col0 + (t + 1) * 128],
                                ident,
                            )
                        xt_sb = xt_pool.tile([128, BLOCK], dt.float16,
                                             tag="xt_sb")
                        if COPY_ACT_EVERY and blk % COPY_ACT_EVERY == 0:
                            nc.scalar.copy(xt_sb, xt_ps)
                        else:
                            nc.vector.tensor_copy(xt_sb, xt_ps)

                        # --- matmuls: [mag | phi_h] then phi_l accumulate ---
                        mm_ps = mm_psum.tile([128, TILES_PER_BLOCK * 256],
                                             dt.float32, tag="mm")
                        for t in range(TILES_PER_BLOCK):
                            lhsT = xt_sb[:, t * 128:(t + 1) * 128]
                            base = t * 256
                            nc.tensor.matmul(
                                mm_ps[:, base:base + 256], lhsT,
                                wcat[:, 0:256], start=True, stop=False)
                            nc.tensor.matmul(
                                mm_ps[:, base + 128:base + 256], lhsT,
                                wcat[:, 256:384], start=False, stop=True)

                        mm3 = mm_ps.rearrange("p (t n) -> p t n", n=256)
                        # --- fused bias + period-1 range reduction (DVE) ---
                        nc.vector._custom_dve(
                            ADD_FRAC_ANT,
                            out=f4[:, gcol0:gcol0 + BLOCK].rearrange(
                                "p (t n) -> p t n", n=128),
                            in0=mm3[:, :, 128:256],
                            in1=brep.rearrange("p (t n) -> p t n", n=128),
                            s0=MAGIC,
                        )
                        # --- magnitude: exp (ACT, psum src) ---
                        nc.scalar.activation(
                            mag4[:, gcol0:gcol0 + BLOCK].rearrange(
                                "p (t n) -> p t n", n=128),
                            mm3[:, :, 0:128], AF.Exp, bias=0.0, scale=1.0)

                    # --- batched sin2pi + multiply over the group ---
                    nc.scalar.activation(cosy4, f4, AF.Sin,
                                         bias=0.0, scale=1.0)  # -> sin2pi
                    if MUL_ON_POOL == 2:
                        eng = nc.gpsimd if (g % 2 == 0) else nc.vector
                    elif MUL_ON_POOL == 1:
                        eng = nc.gpsimd
                    else:
                        eng = nc.vector
                    eng.tensor_mul(
                        out16[:, g * GSZ * BLOCK:(g + 1) * GSZ * BLOCK],
                        mag4, cosy4)

                dst = outd[row0:row0 + SUPER, :].rearrange(
                    "(p r) i -> p r i", p=128)
                nc.gpsimd.dma_start(
                    out=dst, in_=out16.rearrange("p (r i) -> p r i", i=128))
                row0 += SUPER

    nc.compile()

    _real_tjb = nc.to_json_bytes
    nc.to_json_bytes = lambda: _real_tjb().replace(b'"func":"Sin"',
                                                   b'"func":"Sin2pi"')
    _NC_CACHE = nc
    return nc


# --------------------------------------------------------------------------
# Host-side entry point
# --------------------------------------------------------------------------
LAST_RESULT = None


def kernel(x, w_real, w_imag, bias):
    global LAST_RESULT
    x = np.ascontiguousarray(np.asarray(x, dtype=np.float32))
    w_real = np.asarray(w_real, dtype=np.float32)
    w_imag = np.asarray(w_imag, dtype=np.float32)
    bias = np.asarray(bias, dtype=np.float32)

    wr = w_real.astype(np.float64)
    wi = w_imag.astype(np.float64)
    L = 0.5 * np.log(wr * wr + wi * wi)            # [O, I] log|w|
    T = np.arctan2(wi, wr) / (2 * np.pi)           # [O, I] angle in turns
    LT = np.ascontiguousarray(L.T)                 # [I, O]
    TT = np.ascontiguousarray(T.T)                 # [I, O]
    Lh = LT.astype(np.float16)
    Th = TT.astype(np.float16)
    Tl = (TT - Th.astype(np.float64)).astype(np.float16)
    wcat = np.concatenate([Lh, Th, Tl], axis=1)    # [I, 3*O] fp16
    bp = ((bias.astype(np.float64) + np.pi / 2) / (2 * np.pi)).astype(np.float32)
    brep = np.broadcast_to(
        np.tile(bp, BLOCK // O)[None, :], (128, BLOCK)).copy()

    nc = _build_program()

    in_maps = []
    for c in range(N_CORES):
        in_maps.append({
            "x": x[c * B_CORE:(c + 1) * B_CORE, :],
            "wcat": wcat,
            "brep": brep,
        })

    trace = bool(int(os.environ.get("CES_TRACE", "0")))
    if trace:
        _install_trace_shim()
    # Retry once on transient device faults (a wedged NeuronCore usually
    # recovers on the next execute).
    try:
        res = run_bass_kernel_spmd(nc, in_maps, core_ids=list(range(N_CORES)),
                                   trace=trace)
    except Exception:
        import time
        time.sleep(2.0)
        res = run_bass_kernel_spmd(nc, in_maps, core_ids=list(range(N_CORES)),
                                   trace=False)
    LAST_RESULT = res
    if trace and res.exec_time_ns is not None:
        print(f"HW exec time: {res.exec_time_ns} ns", flush=True)

    out = np.empty((B, O), dtype=np.float32)
    for c in range(N_CORES):
        out[c * B_CORE:(c + 1) * B_CORE, :] = res.results[c]["out"]
    return out


def _install_trace_shim():
    """NTFF profiling hook shim (this image's antenv lacks axon_hooks)."""
    import sys
    import types
    import importlib.util as ilu
    if ilu.find_spec("antenv.axon_hooks") is None and \
            "antenv.axon_hooks" not in sys.modules:
        m = types.ModuleType("antenv.axon_hooks")
        h = [None]
        m.set_axon_ntff_profile_hook = lambda v: h.__setitem__(0, v)
        m.get_axon_ntff_profile_hook = lambda: h[0]
        sys.modules["antenv.axon_hooks"] = m
        sys.path.insert(0, "/root/.axon_site")
        try:
            from trn_agent_boot.trn_boot import _ntff_profile_via_ctypes
            m.set_axon_ntff_profile_hook(
                _ntff_profile_via_ctypes("/opt/axon/libaxon_pjrt.so"))
        except Exception:
            pass
    import concourse.bass_utils as bu
    bu.upload_artifacts = lambda d: "local://skipped"


if __name__ == "__main__":
    rng = np.random.default_rng(0)
    x = rng.uniform(-1, 1, (B, I)).astype(np.float32)
    th = rng.uniform(-np.pi, np.pi, (O, I)).astype(np.float32)
    sc = np.exp(0.1 * rng.standard_normal((O, I))).astype(np.float32)
    wr = sc * np.cos(th)
    wi = sc * np.sin(th)
    bs = rng.uniform(-np.pi, np.pi, (O,)).astype(np.float32)
    got = kernel(x, wr, wi, bs)
    absw = np.sqrt(wr.astype(np.float64)**2 + wi.astype(np.float64)**2)
    angw = np.arctan2(wi.astype(np.float64), wr.astype(np.float64))
    mag = np.exp(x.astype(np.float64) @ np.log(absw).T)
    y = x.astype(np.float64) @ angw.T + bs
    ref = np.cos(y) * mag
    err = np.abs(got - ref)
    print(f"absmax={err.max():.3e} scale={np.abs(ref).max():.2f} "
          f"absmax/scale={err.max()/np.abs(ref).max():.3e} "
          f"relL2={np.linalg.norm(got-ref)/np.linalg.norm(ref):.3e}")

